# revision 41
# baseline (speedup 1.0000x reference)
"""GAT (2-layer graph attention) Trainium2 kernel — v3.

Sharding (SPMD, 8 cores): batch b = core//2; within a core pair the 4
attention heads are split 2+2. Pair-local collectives stitch the halves
(AllGather for layer-0 concat, AllReduce(add) for the layer-1 head sum),
each split into two column-halves so the first collective of each layer
hides under the second half's aggregation compute. Tiny static-input
"resync" gathers keep the pair's clocks aligned so the real collectives
absorb almost no rank skew.

HBM/engine highlights: the adjacency mask ships as fp8 (exact for 0/1,
half the DMA bytes) and is widened to bf16 on-chip, with the late tiles
converted inside the aggregation loop so no engine queue head-of-line
blocks on a slow DMA; stage-0/1 matmuls run in bf16; a zero-contribution
"heater" matmul inside each open PSUM accumulation group keeps the PE
activity monitor from down-clocking the tensor engine to 1.2 GHz
mid-aggregation.

All activations live in transposed layout [feature, node]. Both local
heads use the factorized no-ACT attention path: exp(leaky(u)) splits per
branch of the leaky relu,
  P = m.H.e^{ed_i}e^{es_j} + m.(1-H).e^{.2ed_i}e^{.2es_j},
  H = 1{ed_i + es_j >= 0}.
v2 computes the masked indicator in two DVE ops per mask tile —
  Ht = is_ge(edb, -es_j)   (tensor_scalar, 4x bf16 mode)
  G1 = Ht * m              (tensor_tensor, 2x bf16 mode)
— which beats the single 1x-mode scalar_tensor_tensor. Aggregation is
three PE streams per layer (G1_h0, G1_h1 through stacked stationaries
[x.e^{es} | x.e^{.2es}], one shared mask stream through [x2_h0|x2_h1]).
The combine runs in bf16: P3 = ab128 * M128s (2x DVE) and a bf16-moving
signed-selection matmul oT = S128.T @ P3. All reciprocals (layer-0 Z,
layer-1 Z, stage-0 rstd) use the custom-DVE reciprocal_approx_fast, so
the only ACT tables used are exp (+relu/copy) and one sqrt — no
per-instruction table thrash.

Host-side prep (not in HW time): slice x_alpha[:,-1], transpose x, build
per-batch transposed masks, pack/fold weights per core.
"""
import numpy as np
import ml_dtypes
from contextlib import ExitStack

import concourse.bass as bass
import concourse.mybir as mybir
import concourse.tile as tile
from concourse import bacc
from concourse.bass_utils import run_bass_kernel_spmd
from concourse.masks import make_identity

F32 = mybir.dt.float32
BF16 = mybir.dt.bfloat16
AF = mybir.ActivationFunctionType
ALU = mybir.AluOpType

B, T, N, F_IN = 4, 8, 2000, 158
D, H, C = 128, 4, 32
HL = 2             # heads per core
NB = 16            # node blocks (j)
TB = 125           # nodes per block
FA = F_IN + 1      # augmented features (ones col carries b_in)
KA = 128
KB = FA - KA       # 31
CA = C + 1         # 33: head cols + Z column
PW = HL * C + HL   # packed stage-1 cols: x0 (2 heads) + e_src (2 heads)
NEG_SLOPE = 0.2
HALVES = [(0, 1024), (1024, 2000)]
ARH = [(0, 1000), (1000, 2000)]    # AllReduce halves (aligned to TB grid)
CH4 = [(i * 500, (i + 1) * 500) for i in range(4)]

_CACHE = {}


def ts(i, n):
    return slice(i * n, (i + 1) * n)


def _build_program(dumps=False, no_cc=False):
    nc = bacc.Bacc("TRN2", target_bir_lowering=False, debug=False, num_devices=8)

    xTa_d = nc.dram_tensor("xTa", [KA, N], BF16, kind="ExternalInput")
    xTb_d = nc.dram_tensor("xTb", [KB, N], BF16, kind="ExternalInput")
    gT_d = nc.dram_tensor("gT", [N, N], mybir.dt.float8e4,
                          kind="ExternalInput")
    WiaA_d = nc.dram_tensor("WiaA", [KA, D], BF16, kind="ExternalInput")
    WiaB_d = nc.dram_tensor("WiaB", [KB, D], BF16, kind="ExternalInput")
    Wp_d = [nc.dram_tensor(f"Wp{l}", [D, PW], BF16, kind="ExternalInput")
            for l in range(2)]
    Wadb_d = [[nc.dram_tensor(f"Wadb{l}{h}", [D, D], BF16, kind="ExternalInput")
               for h in range(HL)] for l in range(2)]
    lngT_d = nc.dram_tensor("lngT", [D, 1], F32, kind="ExternalInput")
    lnbT_d = nc.dram_tensor("lnbT", [D, 1], F32, kind="ExternalInput")
    b0T_d = nc.dram_tensor("b0T", [D, 1], F32, kind="ExternalInput")
    E44_d = nc.dram_tensor("E44", [4, D], F32, kind="ExternalInput")
    sc128_d = nc.dram_tensor("sc128", [D, 1], F32, kind="ExternalInput")
    S128_d = nc.dram_tensor("S128", [D, CA], BF16, kind="ExternalInput")
    Woa_d = nc.dram_tensor("Woa", [CA, D], BF16, kind="ExternalInput")
    out_d = nc.dram_tensor("out", [N, D], F32, kind="ExternalOutput")
    dbg = {}
    if dumps:
        for nm, shp, dt in (("hT", [D, N], F32), ("hT2", [D, N], F32),
                            ("arin", [C, N], F32)):
            dbg[nm] = nc.dram_tensor(f"dbg_{nm}", shp, dt, kind="ExternalOutput")

    PAIRS = [[0, 1], [2, 3], [4, 5], [6, 7]]

    with tile.TileContext(nc) as tc, ExitStack() as ctx:
        persist = ctx.enter_context(tc.tile_pool(name="persist", bufs=1))
        work = ctx.enter_context(tc.tile_pool(name="work", bufs=2))
        g1p = ctx.enter_context(tc.tile_pool(name="g1p", bufs=2))
        psum = ctx.enter_context(tc.tile_pool(name="ps", bufs=1, space="PSUM"))
        dram = ctx.enter_context(tc.tile_pool(name="dram", bufs=1, space="DRAM"))

        # ---- persistent tiles ----
        ident = persist.tile([128, 128], F32)
        make_identity(nc, ident)
        xTa = persist.tile([KA, N], BF16)
        xTb = persist.tile([KB, N], BF16)
        WiaA = persist.tile([KA, D], BF16)
        WiaB = persist.tile([KB, D], BF16)
        Wp = [persist.tile([D, PW], BF16, name=f"Wp{l}", tag=f"Wp{l}")
              for l in range(2)]
        Wadb = [[persist.tile([D, D], BF16, name=f"Wadb{l}{h}",
                              tag=f"Wadb{l}{h}")
                 for h in range(HL)] for l in range(2)]
        lngT = persist.tile([D, 1], F32)
        lnbT = persist.tile([D, 1], F32)
        b0T = persist.tile([D, 1], F32)
        E44 = persist.tile([4, D], F32)
        sc128 = persist.tile([D, 1], F32)
        S128 = persist.tile([D, CA], BF16)
        Woa = persist.tile([CA, D], BF16)
        ones32 = persist.tile([1, C], F32)
        nc.vector.memset(ones32[:], 1.0)
        eps_t = persist.tile([TB, 1], F32)
        nc.vector.memset(eps_t[:], 1e-5)

        # dram scratch for collectives (per column-half, contiguous)
        warm_in_d = dram.tile([2, 64], F32, tag="warm_in")
        warm_out_d = dram.tile([2, 2, 64], F32, tag="warm_out")
        rs_in_d = [dram.tile([2, 8], F32, tag=f"rs_in{i}", name=f"rs_in{i}")
                   for i in range(2)]
        rs_out_d = [dram.tile([2, 2, 8], F32, tag=f"rs_out{i}",
                              name=f"rs_out{i}") for i in range(2)]
        agf_in_d = [dram.tile([2 * C, hb - ha], BF16, tag=f"agf_in{i}",
                              name=f"agf_in{i}")
                    for i, (ha, hb) in enumerate(HALVES)]
        agf_out_d = [dram.tile([2, 2 * C, hb - ha], BF16, tag=f"agf_out{i}",
                               name=f"agf_out{i}")
                     for i, (ha, hb) in enumerate(HALVES)]
        agz_in_d = [dram.tile([2, hb - ha], F32, tag=f"agz_in{i}",
                              name=f"agz_in{i}")
                    for i, (ha, hb) in enumerate(HALVES)]
        agz_out_d = [dram.tile([2, 2, hb - ha], F32, tag=f"agz_out{i}",
                               name=f"agz_out{i}")
                     for i, (ha, hb) in enumerate(HALVES)]
        ar_in_d = [dram.tile([C, rb - ra], BF16, tag=f"ar_in{i}",
                             name=f"ar_in{i}")
                   for i, (ra, rb) in enumerate(ARH)]
        ar_out_d = [dram.tile([C, rb - ra], BF16, tag=f"ar_out{i}",
                              name=f"ar_out{i}")
                    for i, (ra, rb) in enumerate(ARH)]

        # ---- DMA issue: small weights + chunked xT first, mask spread ----
        for sb, dr in ((WiaA, WiaA_d), (WiaB, WiaB_d), (lngT, lngT_d),
                       (lnbT, lnbT_d)):
            nc.sync.dma_start(out=sb[:], in_=dr[:])
        for q in range(4):
            xq = nc.sync if q % 2 == 0 else nc.scalar
            xq.dma_start(out=xTa[:, ts(q, 500)], in_=xTa_d[:, ts(q, 500)])
        nc.sync.dma_start(out=xTb[:], in_=xTb_d[:])
        for sb, dr in ((Wp[0], Wp_d[0]), (Wp[1], Wp_d[1]),
                       (Wadb[0][0], Wadb_d[0][0]), (Wadb[0][1], Wadb_d[0][1]),
                       (Wadb[1][0], Wadb_d[1][0]), (Wadb[1][1], Wadb_d[1][1]),
                       (b0T, b0T_d), (E44, E44_d), (sc128, sc128_d),
                       (S128, S128_d), (Woa, Woa_d)):
            nc.gpsimd.dma_start(out=sb[:], in_=dr[:])
        if not no_cc:
            # warm the collective path early (first CC trigger pays ~10us)
            nc.gpsimd.collective_compute(
                "AllGather", ALU.bypass, replica_groups=PAIRS,
                ins=[warm_in_d[:].opt()], outs=[warm_out_d[:].opt()])
        # mask ships as fp8 (exact for 0/1, half the HBM bytes) and is
        # widened to bf16 on-chip, split between ACT and DVE.
        gT = persist.tile([TB, NB, N], BF16)
        gt_q = [nc.scalar, nc.gpsimd, nc.scalar, nc.sync, nc.gpsimd,
                nc.scalar, nc.sync, nc.gpsimd, nc.scalar, nc.sync,
                nc.gpsimd, nc.scalar, nc.sync, nc.gpsimd, nc.scalar,
                nc.sync]
        # widen the first half now (these DMAs land early); the rest are
        # emitted inside the agg(0) loop so the ACT queue never head-of-line
        # blocks on a late mask DMA.
        g8L = work.tile([TB, 6, N], mybir.dt.float8e4, tag="g8L", bufs=1)
        for jb in range(10):
            g8 = work.tile([TB, N], mybir.dt.float8e4, tag="g8a", bufs=2)
            gt_q[jb].dma_start(out=g8[:], in_=gT_d[ts(jb, TB), :])
            nc.scalar.copy(gT[:, jb, :], g8[:])
        for jb in range(10, NB):
            gt_q[jb].dma_start(out=g8L[:, jb - 10, :], in_=gT_d[ts(jb, TB), :])

        hT = persist.tile([D, N], F32)
        hTb = persist.tile([D, N], BF16)
        h0T = persist.tile([D, N], BF16)

        # HAM heater: zero stationary/moving for no-op matmuls that keep the
        # PE activity monitor above the downclock threshold.
        z_st = persist.tile([TB, 64], BF16)
        nc.vector.memset(z_st[:], 0.0)
        z_mv = persist.tile([TB, 512], BF16)
        nc.vector.memset(z_mv[:], 0.0)
        hb = psum.tile([64, 512], F32, tag="A0", bufs=1, name="hb")
        for _ in range(28):
            nc.tensor.matmul(hb[:], z_st[:], z_mv[:], start=True, stop=True)

        # per-layer prep tiles (layer-shared: stage1(l+1) depends on the
        # epilogue-updated hT, so reuse is naturally ordered)
        x0ext = persist.tile([TB, NB, HL, CA], BF16)
        xh = [persist.tile([TB, NB, 2 * CA], BF16, name=f"xh{h}",
                           tag=f"xh{h}") for h in range(HL)]
        xg = persist.tile([TB, NB, 97], BF16)
        es_h = persist.tile([TB, NB, HL], F32)
        esn = persist.tile([TB, NB, HL], F32)
        esx = [persist.tile([TB, NB, HL], BF16, name=f"esx{v}",
                            tag=f"esx{v}") for v in range(2)]
        edbBF = [persist.tile([TB, N], BF16, name=f"edb{h}",
                              tag=f"edb{h}") for h in range(HL)]
        ab128 = [persist.tile([D, N], BF16, name=f"ab{h}", tag=f"ab{h}")
                 for h in range(HL)]
        M128s = [persist.tile([D, N], BF16, name=f"M128s{h}", tag=f"M128s{h}")
                 for h in range(HL)]
        arh = persist.tile([CA, N], BF16)  # rows 0:32 AllReduce acc, row 32 ones

        def late_memsets():
            # emitted after stage 0 so they don't head-of-line block the
            # DVE queue ahead of the bn_stats chain
            nc.vector.memset(arh[C:CA, :], 1.0)
            for h in range(HL):
                nc.vector.memset(M128s[h][2 * C:3 * C, :], 0.0)  # rows 66:96
            nc.vector.memset(xg[:, :, C + 1:2 * C], 0.0)
            for jb in range(NB):
                nc.vector.memset(x0ext[:, jb, :, C:CA], 1.0)
        zw4 = persist.tile([4, N], F32)
        rZ4 = persist.tile([4, N], F32)

        # ---- stage 0: input proj + LN + ReLU, batched rstd ----
        hsb = persist.tile([TB, NB, D], BF16)
        stats4 = persist.tile([TB, NB, 6], F32)
        mv4 = persist.tile([TB, NB, 2], F32)
        sd4 = persist.tile([TB, NB], F32)
        rstd4 = persist.tile([TB, NB], F32)
        ph4 = psum.tile([TB, 4, D], F32, tag="A1", bufs=1)
        for nb in range(NB):
            ph = ph4[:, nb % 4, :]
            nc.tensor.matmul(ph, xTa[:, ts(nb, TB)], WiaA[:],
                             start=True, stop=False)
            nc.tensor.matmul(ph, xTb[:, ts(nb, TB)], WiaB[:],
                             start=False, stop=True)
            nc.vector.bn_stats(out=stats4[:, nb, :], in_=ph)
            nc.vector.bn_aggr(out=mv4[:, nb, :], in_=stats4[:, nb, :])
            nc.vector.tensor_copy(hsb[:, nb, :], ph)
        nc.scalar.activation(sd4[:], mv4[:, :, 1:2].rearrange("p n o -> p (n o)"),
                             AF.Sqrt, bias=eps_t[:, 0:1])
        nc.vector.reciprocal_approx_fast(out=rstd4[:], in_=sd4[:])
        identb = persist.tile([TB, TB], BF16)
        nc.vector.tensor_copy(identb[:], ident[0:TB, 0:TB])
        pt4 = psum.tile([D, 4, 128], BF16, tag="Ag", bufs=1)
        for nb in range(NB):
            hn = work.tile([TB, D], BF16, tag="hn", bufs=3)
            nc.vector.tensor_scalar(out=hn[:], in0=hsb[:, nb, :],
                                    scalar1=mv4[:, nb, 0:1],
                                    scalar2=rstd4[:, nb:nb + 1],
                                    op0=ALU.subtract, op1=ALU.mult)
            pt = pt4[:, nb % 4, 0:TB]
            nc.tensor.transpose(pt, hn[:], identb[:])
            nc.scalar.activation(hT[:, ts(nb, TB)], pt, AF.Relu,
                                 scale=lngT[:, 0:1], bias=lnbT[:, 0:1])
            nc.vector.tensor_copy(hTb[:, ts(nb, TB)], hT[:, ts(nb, TB)])
        late_memsets()
        if dumps:
            nc.sync.dma_start(out=dbg["hT"][:], in_=hT[:])

        def stage1_px(l, nb_lo, nb_hi):
            px4 = psum.tile([TB, 4, PW], F32, tag="A1", bufs=1,
                            name=f"px4_{l}_{nb_lo}")
            for nb in range(nb_lo, nb_hi):
                px = px4[:, nb % 4, :]
                nc.tensor.matmul(px, hTb[:, ts(nb, TB)], Wp[l][:],
                                 start=True, stop=True)
                nc.vector.tensor_copy(
                    x0ext[:, nb, :, 0:C],
                    px[:, 0:HL * C].rearrange("p (h c) -> p h c", h=HL))
                nc.vector.tensor_copy(es_h[:, nb, :], px[:, HL * C:PW])
                del px

        def stage1_edb(l, ch_lo, ch_hi):
            for h in range(HL):
                for (ca, cb_) in CH4[ch_lo:ch_hi]:
                    pe = psum.tile([D, 500], F32, tag="misc", bufs=2,
                                   name=f"pe_{l}_{h}_{ca}")
                    nc.tensor.matmul(pe[:], Wadb[l][h][:], hTb[:, ca:cb_],
                                     start=True, stop=True)
                    nc.vector.tensor_copy(edbBF[h][:, ca:cb_], pe[0:TB, :])
                    nc.scalar.activation(ab128[h][:, ca:cb_], pe[:],
                                         AF.Exp, scale=sc128[:, 0:1])

        def stage1_esn(l):
            esv = es_h[:].rearrange("p nb h -> p (nb h)")
            nc.vector.tensor_scalar(out=esn[:].rearrange("p nb h -> p (nb h)"),
                                    in0=esv, scalar1=-1.0, scalar2=None,
                                    op0=ALU.mult)

        def stage1_fin(l):
            """exp factors + stationaries (needs all px done)."""
            esv = es_h[:].rearrange("p nb h -> p (nb h)")
            nc.scalar.activation(esx[0][:].rearrange("p nb h -> p (nb h)"),
                                 esv, AF.Exp)
            nc.scalar.activation(esx[1][:].rearrange("p nb h -> p (nb h)"),
                                 esv, AF.Exp, scale=NEG_SLOPE)
            # xh = [x0*e^es | x0*e^.2es]; xg = [x2_h0 | x2_h1]
            for h in range(HL):
                xv = xh[h][:].rearrange("p nb (v c) -> p nb v c", v=2)
                for v in range(2):
                    src = esx[v][:, :, h:h + 1]
                    bcast = bass.AP(tensor=src.tensor, offset=src.offset,
                                    ap=[src.ap[0], src.ap[1], [0, CA]])
                    nc.vector.tensor_tensor(out=xv[:, :, v, :],
                                            in0=x0ext[:, :, h, :], in1=bcast,
                                            op=ALU.mult)
            for h in range(HL):
                # xg97 cols per head: feats x2 at 64h..64h+32, zg col at 32+64h
                nc.scalar.copy(xg[:, :, 64 * h:64 * h + C],
                               xh[h][:, :, CA:CA + C])
                nc.scalar.copy(xg[:, :, C + 64 * h:C + 64 * h + 1],
                               xh[h][:, :, CA + C:2 * CA])

        def stage1(l):
            # order: px -> esn -> edb (what agg's first G1 needs) before the
            # xh/xg packing, so the aggregation can start sooner
            stage1_px(l, 0, NB)
            stage1_esn(l)
            stage1_edb(l, 0, len(CH4))
            stage1_fin(l)

        def agg(l, tail, cc_launch):
            """Aggregation over j in i-halves; tail(h, a, b, po, Zs)."""
            for hi, (ha, hb_) in enumerate(HALVES):
                hw = hb_ - ha
                M12 = [psum.tile([2 * CA, 1024], F32, tag=f"A{h}", bufs=1,
                                 name=f"M12_{h}") for h in range(HL)]
                Mg = psum.tile([97, 1024], F32, tag="Ag", bufs=1)
                subs = [(0, 512), (512, hw)]
                for jb in range(NB):
                    if l == 0 and hi == 0 and jb < 6:
                        nc.scalar.copy(gT[:, 10 + jb, :], g8L[:, jb, :])
                    for h in range(HL):
                        Ht = g1p.tile([TB, 1024], BF16, tag="ht")
                        nc.vector.tensor_scalar(
                            out=Ht[:, 0:hw], in0=edbBF[h][:, ha:hb_],
                            scalar1=esn[:, jb, h:h + 1], scalar2=None,
                            op0=ALU.is_ge)
                        G1 = g1p.tile([TB, 1024], BF16, tag="g1", bufs=2)
                        nc.vector.tensor_tensor(
                            out=G1[:, 0:hw], in0=Ht[:, 0:hw],
                            in1=gT[:, jb, ha:hb_], op=ALU.mult)
                        for (sa, sb_) in subs:
                            nc.tensor.matmul(M12[h][:, sa:sb_],
                                             xh[h][:, jb, :], G1[:, sa:sb_],
                                             start=(jb == 0),
                                             stop=(jb == NB - 1))
                        if h == 0 and 0 < jb < NB - 1:
                            # heater: += 0 into the open accumulation group
                            nc.tensor.matmul(M12[h][0:64, 0:512],
                                             z_st[:], z_mv[:],
                                             start=False, stop=False)
                    for (sa, sb_) in subs:
                        nc.tensor.matmul(Mg[:, sa:sb_], xg[:, jb, :],
                                         gT[:, jb, ha + sa:ha + sb_],
                                         start=(jb == 0),
                                         stop=(jb == NB - 1))
                for h in range(HL):
                    # M128s rows: 0:66 = [M1|Z1|M2|Z2], 66 = Zg, 96:128 =
                    # Mg feats. sc128[66]=0.2 and S128[66,32]=+1 fold the
                    # a2*Zg term straight into the combine matmul, so
                    # po[C:CA] is the complete softmax normalizer Z.
                    nc.scalar.copy(M128s[h][0:2 * CA, ha:hb_], M12[h][:, 0:hw])
                    nc.scalar.copy(M128s[h][3 * C:D, ha:hb_],
                                   Mg[64 * h:64 * h + C, 0:hw])
                    # ACT can't write partition base 66 (32-align rule);
                    # stage the Zg row at base 0 and DMA it into place.
                    zgs = work.tile([1, 1024], BF16, tag="zgs", bufs=1)
                    nc.scalar.copy(zgs[:, 0:hw],
                                   Mg[C + 64 * h:C + 64 * h + 1, 0:hw])
                    nc.scalar.dma_start(out=M128s[h][66:67, ha:hb_],
                                        in_=zgs[:, 0:hw])
                    for (sa, sb_) in subs:
                        P3 = work.tile([D, 512], BF16, tag="P3", bufs=2)
                        nc.vector.tensor_tensor(
                            out=P3[:, 0:sb_ - sa],
                            in0=ab128[h][:, ha + sa:ha + sb_],
                            in1=M128s[h][:, ha + sa:ha + sb_], op=ALU.mult)
                        po = psum.tile([CA, 512], F32, tag="misc", bufs=2)
                        nc.tensor.matmul(po[:, 0:sb_ - sa], S128[:],
                                         P3[:, 0:sb_ - sa], start=True,
                                         stop=True)
                        tail(h, ha, ha + sa, ha + sb_, po)
                if hi == 0 and not no_cc:
                    # tiny pair-resync mid-agg: absorbs accumulated rank
                    # skew while both cores still have compute in flight
                    nc.gpsimd.collective_compute(
                        "AllGather", ALU.bypass, replica_groups=PAIRS,
                        ins=[rs_in_d[l][:].opt()],
                        outs=[rs_out_d[l][:].opt()])
                cc_launch(hi)

        # ================= layer 0 =================
        stage1(0)

        def tail0(h, ha, a, b_, po):
            hi = 0 if ha == 0 else 1
            w = b_ - a
            Zs = work.tile([1, 512], F32, tag="Zs", bufs=2)
            nc.scalar.copy(Zs[:, 0:w], po[C:CA, 0:w])
            nc.scalar.dma_start(out=agz_in_d[hi][h:h + 1, a - ha:b_ - ha],
                                in_=Zs[:, 0:w])
            fTs = work.tile([C, 512], BF16, tag="fTs", bufs=2)
            nc.scalar.copy(fTs[:, 0:w], po[0:C, 0:w])
            nc.scalar.dma_start(out=agf_in_d[hi][ts(h, C), a - ha:b_ - ha],
                                in_=fTs[0:C, 0:w])

        def cc0(hi):
            if no_cc:
                nc.gpsimd.dma_start(out=agz_out_d[hi][0], in_=agz_in_d[hi][:])
                nc.gpsimd.dma_start(out=agz_out_d[hi][1], in_=agz_in_d[hi][:])
                nc.gpsimd.dma_start(out=agf_out_d[hi][0], in_=agf_in_d[hi][:])
                nc.gpsimd.dma_start(out=agf_out_d[hi][1], in_=agf_in_d[hi][:])
            else:
                nc.gpsimd.collective_compute(
                    "AllGather", ALU.bypass, replica_groups=PAIRS,
                    ins=[agz_in_d[hi][:].opt()], outs=[agz_out_d[hi][:].opt()])
                nc.gpsimd.collective_compute(
                    "AllGather", ALU.bypass, replica_groups=PAIRS,
                    ins=[agf_in_d[hi][:].opt()], outs=[agf_out_d[hi][:].opt()])

        agg(0, tail0, cc0)

        # h = h + elu(h0/Z + bias0), all in T layout; process chunks 0-1
        # (cols 0:1000 ⊂ gather half 0) before touching half 1 so the DVE
        # queue never head-of-line blocks on the second collective.
        def epi_gather(hi):
            ha, hb_ = HALVES[hi]
            nc.sync.dma_start(out=zw4[0:2, ha:hb_], in_=agz_out_d[hi][0])
            nc.sync.dma_start(out=zw4[2:4, ha:hb_], in_=agz_out_d[hi][1])
            nc.vector.reciprocal_approx_fast(out=rZ4[:, ha:hb_],
                                             in_=zw4[:, ha:hb_])
            nc.sync.dma_start(out=h0T[0:2 * C, ha:hb_], in_=agf_out_d[hi][0])
            nc.sync.dma_start(out=h0T[2 * C:D, ha:hb_], in_=agf_out_d[hi][1])

        def epi_chunk(ca, cb_):
            rzb = psum.tile([D, 500], F32, tag="misc", bufs=2)
            nc.tensor.matmul(rzb[:], E44[:], rZ4[:, ca:cb_], start=True,
                             stop=True)
            u0 = work.tile([D, 500], F32, tag="u0", bufs=2)
            nc.vector.tensor_tensor(out=u0[:], in0=h0T[:, ca:cb_], in1=rzb[:],
                                    op=ALU.mult)
            nc.vector.tensor_scalar(out=u0[:], in0=u0[:], scalar1=b0T[:, 0:1],
                                    scalar2=None, op0=ALU.add)
            r2 = work.tile([D, 500], F32, tag="r2", bufs=2)
            nc.scalar.activation(r2[:], u0[:], AF.Relu, scale=-1.0)
            ex = work.tile([D, 500], F32, tag="ex", bufs=2)
            nc.scalar.activation(ex[:], r2[:], AF.Exp, scale=-1.0)
            # hT += u0 + relu(-u0) + (e^{-relu(-u0)} - 1)  ==  hT += elu(u0)
            nc.gpsimd.tensor_tensor(out=r2[:], in0=r2[:], in1=u0[:],
                                    op=ALU.add)
            nc.vector.scalar_tensor_tensor(out=ex[:], in0=ex[:],
                                           scalar=-1.0, in1=r2[:],
                                           op0=ALU.add, op1=ALU.add)
            nc.gpsimd.tensor_tensor(out=hT[:, ca:cb_], in0=hT[:, ca:cb_],
                                    in1=ex[:], op=ALU.add)
            nc.vector.tensor_copy(hTb[:, ca:cb_], hT[:, ca:cb_])

        epi_gather(0)
        epi_chunk(*CH4[0])
        epi_chunk(*CH4[1])
        epi_gather(1)
        epi_chunk(*CH4[2])
        epi_chunk(*CH4[3])
        if dumps:
            nc.sync.dma_start(out=dbg["hT2"][:], in_=hT[:])

        # ================= layer 1 =================
        stage1(1)

        def tail1(h, ha, a, b_, po):
            w = b_ - a
            Z1s = work.tile([1, 512], F32, tag="Z1s", bufs=2)
            nc.scalar.copy(Z1s[:, 0:w], po[C:CA, 0:w])
            rZ1 = work.tile([1, 512], F32, tag="rZ1", bufs=2)
            nc.vector.reciprocal_approx_fast(out=rZ1[:, 0:w],
                                             in_=Z1s[:, 0:w])
            oTs = work.tile([C, 512], BF16, tag="oTs", bufs=2)
            nc.scalar.copy(oTs[:, 0:w], po[0:C, 0:w])
            rzb1 = psum.tile([C, 512], F32, tag="misc", bufs=2)
            nc.tensor.matmul(rzb1[:, 0:w], ones32[:], rZ1[:, 0:w],
                             start=True, stop=True)
            if h == 0:
                nc.vector.tensor_tensor(out=arh[0:C, a:b_],
                                        in0=oTs[:, 0:w],
                                        in1=rzb1[:, 0:w], op=ALU.mult)
            else:
                c1 = work.tile([C, 512], BF16, tag="c1b", bufs=2)
                nc.vector.tensor_tensor(out=c1[:, 0:w], in0=oTs[:, 0:w],
                                        in1=rzb1[:, 0:w], op=ALU.mult)
                nc.vector.tensor_tensor(out=arh[0:C, a:b_],
                                        in0=arh[0:C, a:b_],
                                        in1=c1[:, 0:w], op=ALU.add)

        def cc1(hi):
            # ARH half 0 (cols 0:1000) is complete after agg half 0 — its
            # AllReduce hides under agg half 1; half 1 launches at the end.
            (ra, rb_) = ARH[hi]
            nc.scalar.dma_start(out=ar_in_d[hi][:], in_=arh[0:C, ra:rb_])
            if dumps:
                nc.gpsimd.dma_start(out=dbg["arin"][:, ra:rb_],
                                    in_=arh[0:C, ra:rb_])
            if no_cc:
                nc.gpsimd.dma_start(out=ar_out_d[hi][:], in_=ar_in_d[hi][:])
            else:
                nc.gpsimd.collective_compute(
                    "AllReduce", ALU.add, replica_groups=PAIRS,
                    ins=[ar_in_d[hi][:].opt()], outs=[ar_out_d[hi][:].opt()])

        agg(1, tail1, cc1)

        # final projection: out = [h1_sum | 1] @ [W_out/4 ; bias1@W_out+b_out]
        oq = [nc.sync, nc.scalar, nc.gpsimd]
        for hi, (ra, rb_) in enumerate(ARH):
            nc.sync.dma_start(out=arh[0:C, ra:rb_], in_=ar_out_d[hi][:])
            pf4 = psum.tile([TB, 4, D], F32, tag="A1", bufs=1,
                            name=f"pf4_{hi}")
            for nb in range(8 * hi, 8 * (hi + 1)):
                po = pf4[:, nb % 4, :]
                nc.tensor.matmul(po, arh[:, ts(nb, TB)], Woa[:], start=True,
                                 stop=True)
                ob = work.tile([TB, D], F32, tag="ob", bufs=3)
                nc.scalar.copy(ob[:], po)
                oq[nb % 3].dma_start(out=out_d[ts(nb, TB), :], in_=ob[:])

    nc.compile()
    return nc


def _host_prep(inputs):
    """Build the 8 per-core input maps (pure numpy, not in HW time)."""
    f32 = np.float32
    x = np.asarray(inputs["x_alpha"], f32)[:, -1]            # [B, N, F_IN]
    sg = np.asarray(inputs["sector_graph"], f32)
    W_in = np.asarray(inputs["W_in"], f32)
    b_in = np.asarray(inputs["b_in"], f32)
    ln_g = np.asarray(inputs["ln_g"], f32)
    ln_b = np.asarray(inputs["ln_b"], f32)
    W0 = np.asarray(inputs["W0"], f32)
    as0 = np.asarray(inputs["as0"], f32)
    ad0 = np.asarray(inputs["ad0"], f32)
    bias0 = np.asarray(inputs["bias0"], f32)
    W1 = np.asarray(inputs["W1"], f32)
    as1 = np.asarray(inputs["as1"], f32)
    ad1 = np.asarray(inputs["ad1"], f32)
    bias1 = np.asarray(inputs["bias1"], f32)
    W_out = np.asarray(inputs["W_out"], f32)
    b_out = np.asarray(inputs["b_out"], f32)

    Wia = np.concatenate([W_in, b_in[None, :]], axis=0)       # [159, 128]
    Woa = np.concatenate([W_out / H, (bias1 @ W_out + b_out)[None, :]], axis=0)
    E44 = np.zeros((4, D), f32)
    for g in range(4):
        E44[g, g * C:(g + 1) * C] = 1.0
    CA_ = C + 1
    sc128 = np.zeros((D, 1), f32)
    sc128[0:CA_] = 1.0
    sc128[CA_:2 * CA_] = 0.2
    sc128[2 * CA_:2 * CA_ + 1] = 0.2   # row 66: a2 factor for the Zg row
    sc128[3 * C:D] = 0.2
    S128 = np.zeros((D, CA_), f32)
    for r in range(CA_):
        S128[r, r] = 1.0            # a1*M1 block (+Z1 at 32)
        S128[CA_ + r, r] = -1.0     # a2*M2 block (-Z2)
    S128[2 * CA_, C] = 1.0          # row 66 -> Z col: +a2*Zg
    for r in range(C):
        S128[3 * C + r, r] = 1.0    # a2*Mg feats block

    eye = np.eye(N, dtype=bool)
    in_maps = []
    for c in range(8):
        b = c // 2
        hp = c % 2
        heads = [2 * hp, 2 * hp + 1]
        xT = np.ascontiguousarray(x[b].T)                      # [158, N]
        # augmented [x | 1] transposed is [159, N]; split 128 + 31 rows
        xTfull = np.concatenate([xT, np.ones((1, N), f32)], axis=0)  # [159,N]
        xTa = np.ascontiguousarray(xTfull[0:KA])
        xTb = np.ascontiguousarray(xTfull[KA:FA])              # [31, N]
        mask = (sg[b] > 0) | eye
        gT = np.ascontiguousarray(mask.T).astype(np.float32)

        def pack_p(W, as_):
            cols = [W[:, h * C:(h + 1) * C] for h in heads]
            cols += [(W[:, h * C:(h + 1) * C] @ as_[h])[:, None] for h in heads]
            return np.ascontiguousarray(np.concatenate(cols, axis=1))

        def pack_adb(W, ad_, h):
            v = W[:, h * C:(h + 1) * C] @ ad_[h]               # [D]
            return np.ascontiguousarray(np.tile(v[:, None], (1, D)))

        bf = ml_dtypes.bfloat16
        in_maps.append({
            "xTa": xTa.astype(bf), "xTb": xTb.astype(bf),
            "gT": gT.astype(ml_dtypes.float8_e4m3fn),
            "WiaA": np.ascontiguousarray(Wia[0:KA]).astype(bf),
            "WiaB": np.ascontiguousarray(Wia[KA:FA]).astype(bf),
            "Wp0": pack_p(W0, as0).astype(bf), "Wp1": pack_p(W1, as1).astype(bf),
            "Wadb00": pack_adb(W0, ad0, heads[0]).astype(bf),
            "Wadb01": pack_adb(W0, ad0, heads[1]).astype(bf),
            "Wadb10": pack_adb(W1, ad1, heads[0]).astype(bf),
            "Wadb11": pack_adb(W1, ad1, heads[1]).astype(bf),
            "lngT": np.ascontiguousarray(ln_g[:, None]),
            "lnbT": np.ascontiguousarray(ln_b[:, None]),
            "b0T": np.ascontiguousarray(bias0[:, None]),
            "E44": E44,
            "sc128": sc128,
            "S128": S128.astype(ml_dtypes.bfloat16),
            "Woa": np.ascontiguousarray(Woa).astype(bf),
        })
    return in_maps


def kernel(**inputs):
    if "nc" not in _CACHE:
        _CACHE["nc"] = _build_program()
    nc = _CACHE["nc"]
    in_maps = _host_prep(inputs)
    res = run_bass_kernel_spmd(nc, in_maps, list(range(8)),
                               **_CACHE.get("run_kwargs", {}))
    _CACHE["last_results"] = res
    out = np.empty((B, N, D), np.float32)
    for b in range(B):
        out[b] = res.results[2 * b]["out"]
    return out
